# revision 15
# baseline (speedup 1.0000x reference)
"""Trainium2 Bass kernel for nn_AttentionModel (dense transformer MHA fwd).

Reference math (per batch b):
  q = x_q @ Wq.T + bq ; k,v likewise     (S=2048, E=1024, H=16, Dh=64)
  scores = q @ k.T  (per head)
  scores[sk where attn_mask[b,sk]==0] = -inf
  attn = softmax(scores, -1) * dropout_mask[b,h]
  out = attn @ v                          -> (B, H, S, Dh)

Sharding: 8 cores = 2 batches x 4 head-groups (4 heads/core). Pure data
parallel SPMD, no collectives; host slices inputs and restacks outputs.

v3 dataflow (fully transposed attention, K=128 everywhere):
  - Host pre-transposes x/W (f16) and dm (bf16, [h, g, sk, 512] layout);
    halves HBM traffic, no device-side transposes of x/W/dm needed.
  - Projections produce qT2 head-PAIR tensors [128, S] (head a rows 0-63,
    head b rows 64-127) straight from PSUM, and per-head half-zero kT2
    tensors; scores^T = kT2_h (stationary, K=128: zero rows contribute 0)
    @ qT2_p (moving). K=128 avoids the half-rate small-K PE mode.
  - Key mask applied as per-partition bias in the exp activation (sk is
    the partition dim of scores^T): masked rows exp to exactly 0.
  - PDMT = EmT * dmT elementwise, split across DVE and Pool engines.
  - attn@v accumulates out^T[d, sq] over sk tiles (v natural stationary);
    Z = colsum(EmT) via a ones-column matmul PSUM chain; 1/Z applied to
    out^T via an fp32r broadcast matmul + one DVE mul. Output is written
    to DRAM transposed [Dh, S]; the host restores [S, Dh].
"""

import numpy as np

S = 2048
E = 1024
H_TOT = 16
NH = 4  # heads per core
Dh = 64
B = 2
N_CORES = 8
ST = S // 128  # 16 sk-tiles
ET = E // 128  # 8 e-tiles
SCH = 4  # s-chunks of 512 for the projection phase
GROUPS = 4  # sq groups of 512
EXP_SHIFT = -12.0  # exp(s + EXP_SHIFT)
MASK_BIG = 60000.0
DM_FIX = 1.0015650      # (1/0.9) / bf16(1/0.9): dm is cast to bf16 on host

_CACHE = {}


def _build_program():
    import concourse.bacc as bacc
    import concourse.bass as bass
    import concourse.mybir as mybir
    import concourse.tile as tile
    from concourse.masks import make_identity
    from contextlib import ExitStack

    dt = mybir.dt
    F32 = dt.float32
    F32R = dt.float32r
    F16 = dt.float16
    BF16 = dt.bfloat16
    I32 = dt.int32

    nc = bacc.Bacc("TRN2", target_bir_lowering=False, debug=False)

    # host-pretransposed inputs
    xq_d = nc.dram_tensor("xqT", [E, S], F16, kind="ExternalInput")
    xk_d = nc.dram_tensor("xkT", [E, S], F16, kind="ExternalInput")
    xv_d = nc.dram_tensor("xvT", [E, S], F16, kind="ExternalInput")
    wq_d = nc.dram_tensor("wqT", [E, NH * Dh], F16, kind="ExternalInput")
    wk_d = nc.dram_tensor("wkT", [E, NH * Dh], F16, kind="ExternalInput")
    wv_d = nc.dram_tensor("wvT", [E, NH * Dh], F16, kind="ExternalInput")
    bq_d = nc.dram_tensor("bq", [NH * Dh], F32, kind="ExternalInput")
    bk_d = nc.dram_tensor("bk", [NH * Dh], F32, kind="ExternalInput")
    bv_d = nc.dram_tensor("bv", [NH * Dh], F32, kind="ExternalInput")
    am_d = nc.dram_tensor("amask", [S], I32, kind="ExternalInput")
    # dmT[h, g, sk, j] = dm[h, g*512 + j, sk]
    dm_d = nc.dram_tensor("dmT", [NH, GROUPS, S, 512], BF16, kind="ExternalInput")
    out_d = nc.dram_tensor("out", [NH, S, Dh], F32, kind="ExternalOutput")

    def r32(ap):
        return ap.bitcast(F32R)

    with tile.TileContext(nc) as tc, ExitStack() as ctx:
        const_pool = ctx.enter_context(tc.tile_pool(name="const", bufs=1))

        # --- attn_mask -> per-partition exp bias columns mb16[p, i] ---
        m_colI = const_pool.tile([128, ST], I32)
        nc.sync.dma_start(m_colI[:], am_d[:].rearrange("(t p) -> p t", p=128))
        m_col = const_pool.tile([128, ST], F32)
        nc.vector.tensor_copy(m_col[:], m_colI[:])
        # mb16 = m*BIG + (EXP_SHIFT - BIG): 0 -> -BIG+shift, 1 -> shift
        mb16 = const_pool.tile([128, ST], F32)
        nc.scalar.activation(
            mb16[:], m_col[:], mybir.ActivationFunctionType.Copy,
            bias=EXP_SHIFT - MASK_BIG, scale=MASK_BIG,
        )

        # --- bias columns for q/k pair evac; bv broadcast row; ones ---
        bqp = []
        bkp = []
        for p in range(2):
            t = const_pool.tile([128, 1], F32, tag=f"bqp{p}", name=f"bqp{p}")
            nc.sync.dma_start(t[:], bq_d[p * 128:(p + 1) * 128].rearrange("(c o) -> c o", o=1))
            bqp.append(t)
            t = const_pool.tile([128, 1], F32, tag=f"bkp{p}", name=f"bkp{p}")
            nc.sync.dma_start(t[:], bk_d[p * 128:(p + 1) * 128].rearrange("(c o) -> c o", o=1))
            bkp.append(t)
        ones_row = const_pool.tile([1, 128], F32)
        nc.gpsimd.memset(ones_row[:], 1.0)
        ones_col = const_pool.tile([128, 1], BF16)
        nc.gpsimd.memset(ones_col[:], 1.0)
        one_1x1 = const_pool.tile([1, 1], F32)
        nc.gpsimd.memset(one_1x1[:], 1.0)
        ident = const_pool.tile([64, 64], F32)
        make_identity(nc, ident[:])
        bv_row = const_pool.tile([1, NH * Dh], F32)
        nc.sync.dma_start(bv_row[:], bv_d[:].rearrange("(o c) -> o c", o=1))
        bv_bc = const_pool.tile([128, NH * Dh], F32)

        with tc.tile_pool(name="ps_misc", bufs=1, space="PSUM") as ps_misc:
            bc_ps = ps_misc.tile([128, NH * Dh], F32)
            nc.tensor.matmul(bc_ps[:], ones_row[:], bv_row[:])
            nc.scalar.mul(bv_bc[:], bc_ps[:], DM_FIX)

        # --- persistent attention-phase tensors ---
        big_pool = ctx.enter_context(tc.tile_pool(name="big", bufs=1))
        # qT2[p]: head pair p, rows 0-63 = head 2p, rows 64-127 = head 2p+1
        qT2 = [big_pool.tile([128, S], F16, tag=f"qT2{p}", name=f"qT2{p}")
               for p in range(2)]
        # kT2[h]: [128, S] with only this head's 64 rows nonzero
        kT2 = [big_pool.tile([128, S], F16, tag=f"kT2{h}", name=f"kT2{h}")
               for h in range(NH)]
        v16 = big_pool.tile([128, ST, NH * Dh], BF16)

        # zero the unused halves of kT2 (once)
        for h in range(NH):
            if h % 2 == 0:
                nc.gpsimd.memset(kT2[h][64:128, :], 0.0)
            else:
                nc.gpsimd.memset(kT2[h][0:64, :], 0.0)

        # ============ Phase 2: projections ============
        with tc.tile_pool(name="wt", bufs=1) as wtp, \
             tc.tile_pool(name="xT", bufs=3) as xtp, \
             tc.tile_pool(name="ps_prj", bufs=2, space="PSUM") as ps_prj, \
             tc.tile_pool(name="ps_prv", bufs=2, space="PSUM") as ps_prv:

            wts = {}
            for name, w_d in (("q", wq_d), ("k", wk_d), ("v", wv_d)):
                wt = wtp.tile([128, ET, NH * Dh], F16, tag=f"wt_{name}",
                              name=f"wt_{name}")
                nc.gpsimd.dma_start(
                    wt[:], w_d[:].rearrange("(et p) m -> p et m", p=128))
                wts[name] = wt

            for tens, x_d in (("v", xv_d), ("k", xk_d), ("q", xq_d)):
                for sc in range(SCH):
                    xt_c = xtp.tile([128, ET, 512], F16, tag="xt")
                    nc.gpsimd.dma_start(
                        xt_c[:],
                        x_d[:, sc * 512:(sc + 1) * 512].rearrange(
                            "(et p) n -> p et n", p=128))
                    sl = slice(sc * 512, (sc + 1) * 512)

                    if tens in ("q", "k"):
                        wt = wts[tens]
                        bias = bqp if tens == "q" else bkp
                        for p in range(2):
                            pq = ps_prj.tile([128, 512], F32, tag="pqk")
                            for et in range(ET):
                                nc.tensor.matmul(
                                    pq[:],
                                    wt[:, et, p * 128:(p + 1) * 128],
                                    xt_c[:, et, :],
                                    start=(et == 0), stop=(et == ET - 1))
                            if tens == "q":
                                nc.scalar.activation(
                                    qT2[p][:, sl], pq[:],
                                    mybir.ActivationFunctionType.Identity,
                                    bias=bias[p][:])
                            else:
                                nc.scalar.activation(
                                    kT2[2 * p][0:64, sl], pq[0:64, :],
                                    mybir.ActivationFunctionType.Identity,
                                    bias=bias[p][0:64, :])
                                nc.scalar.activation(
                                    kT2[2 * p + 1][64:128, sl], pq[64:128, :],
                                    mybir.ActivationFunctionType.Identity,
                                    bias=bias[p][64:128, :])
                    else:
                        for st in range(4):
                            pv = ps_prv.tile([128, NH * Dh], F32, tag="pv")
                            for et in range(ET):
                                nc.tensor.matmul(
                                    pv[:],
                                    xt_c[:, et, st * 128:(st + 1) * 128],
                                    wts["v"][:, et, :],
                                    start=(et == 0), stop=(et == ET - 1))
                            nc.vector.scalar_tensor_tensor(
                                out=v16[:, sc * 4 + st, :], in0=pv[:],
                                scalar=DM_FIX, in1=bv_bc[:],
                                op0=mybir.AluOpType.mult,
                                op1=mybir.AluOpType.add)

        # ============ Phase 3: attention (transposed, K=128) ============
        with tc.tile_pool(name="dmT", bufs=5) as dmp, \
             tc.tile_pool(name="em", bufs=10) as emp, \
             tc.tile_pool(name="pdmt", bufs=8) as pdmtp, \
             tc.tile_pool(name="rz", bufs=4) as rzp, \
             tc.tile_pool(name="ot", bufs=2) as otp, \
             tc.tile_pool(name="ostage", bufs=2) as ostp, \
             tc.tile_pool(name="ps_s", bufs=4, space="PSUM") as ps_s, \
             tc.tile_pool(name="ps_av", bufs=4, space="PSUM") as ps_av:

            for h in range(NH):
                p = h // 2
                ost = ostp.tile([128, ST * Dh], F32, tag="ost")
                subs = []
                for g in range(GROUPS):
                    dmt = dmp.tile([128, ST, 512], BF16, tag="dmt")
                    nc.sync.dma_start(
                        dmt[:],
                        dm_d[h, g].rearrange("(t p) j -> p t j", p=128))
                    # av rows 0-63 = out^T chain; row 64 = Z chain
                    av65 = ps_av.tile([65, 512], F32, tag="av")
                    subs.append((g, dmt, av65))

                # four interleaved dependency streams keep the PE fed
                for i in range(ST):
                    for g, dmt, av65 in subs:
                        sp = ps_s.tile([128, 512], F32, tag="sps")
                        nc.tensor.matmul(
                            sp[:],
                            kT2[h][:, i * 128:(i + 1) * 128],
                            qT2[p][:, g * 512:(g + 1) * 512])
                        em = emp.tile([128, 512], BF16, tag="em")
                        nc.scalar.activation(
                            em[:], sp[:],
                            mybir.ActivationFunctionType.Exp,
                            bias=mb16[:, i:i + 1])
                        pdmt = pdmtp.tile([128, 512], BF16, tag="pdmt")
                        eng = nc.gpsimd if (i % 5) < 2 else nc.vector
                        eng.tensor_mul(pdmt[:], em[:], dmt[:, i, :])
                        nc.tensor.matmul(
                            av65[0:64, :],
                            v16[:, i, h * Dh:(h + 1) * Dh],
                            pdmt[:],
                            start=(i == 0), stop=(i == ST - 1),
                            skip_group_check=True)
                        nc.tensor.matmul(
                            av65[64:65, :],
                            ones_col[:],
                            em[:],
                            start=(i == 0), stop=(i == ST - 1),
                            skip_group_check=True)

                for g, dmt, av65 in subs:
                    # Z row -> column form [128, 4] via K=1 matmuls that
                    # accumulate into one zeroed PSUM region
                    zrowS = rzp.tile([1, 512], F32, tag="zrowS")
                    nc.vector.tensor_copy(zrowS[:], av65[64:65, :])
                    zc4 = ps_s.tile([128, 512], F32, tag="sps")
                    for c in range(4):
                        nc.tensor.matmul(
                            zc4[:, c:c + 1],
                            zrowS[0:1, c * 128:(c + 1) * 128], one_1x1[:],
                            start=(c == 0), stop=(c == 3),
                            skip_group_check=True)
                    rz4 = rzp.tile([128, 4], F32, tag="rz4")
                    nc.vector.reciprocal(rz4[:], zc4[:, 0:4])
                    # out^T -> natural + 1/Z scale
                    ot64 = otp.tile([64, 512], F32, tag="ot64")
                    nc.vector.tensor_copy(ot64[:], av65[0:64, :])
                    on4 = ps_s.tile([128, 512], F32, tag="sps")
                    for c in range(4):
                        nc.tensor.matmul(
                            on4[:, c * 64:(c + 1) * 64],
                            ot64[:, c * 128:(c + 1) * 128], ident[:],
                            is_transpose=True,
                            start=(c == 0), stop=(c == 3),
                            skip_group_check=True)
                    for c in range(4):
                        nc.vector.tensor_scalar_mul(
                            ost[:, g * 256 + c * 64:g * 256 + (c + 1) * 64],
                            on4[:, c * 64:(c + 1) * 64], rz4[:, c:c + 1])
                nc.sync.dma_start(
                    out_d[h].rearrange("(t p) d -> p t d", p=128), ost[:])

    nc.compile()
    return nc


def _get_program():
    if "nc" not in _CACHE:
        _CACHE["nc"] = _build_program()
    return _CACHE["nc"]


def make_in_maps(query, key, value, attn_mask, dropout_mask, Wq, bq, Wk, bk, Wv, bv):
    import ml_dtypes
    BF = ml_dtypes.bfloat16

    xT = {}
    for b in range(B):
        xT[("q", b)] = np.ascontiguousarray(query[b].T.astype(np.float16))
        xT[("k", b)] = np.ascontiguousarray(key[b].T.astype(np.float16))
        xT[("v", b)] = np.ascontiguousarray(value[b].T.astype(np.float16))
    # dmT[b][h, g, sk, j] = dm[b, h, g*512+j, sk]
    dmT_all = np.ascontiguousarray(
        dropout_mask.reshape(B, H_TOT, GROUPS, 512, S).transpose(0, 1, 2, 4, 3)
        .astype(BF))

    in_maps = []
    for c in range(N_CORES):
        b = c // 4
        h0 = (c % 4) * NH
        rs = slice(h0 * Dh, (h0 + NH) * Dh)
        in_maps.append({
            "xqT": xT[("q", b)],
            "xkT": xT[("k", b)],
            "xvT": xT[("v", b)],
            "wqT": np.ascontiguousarray(Wq[rs].T.astype(np.float16)),
            "wkT": np.ascontiguousarray(Wk[rs].T.astype(np.float16)),
            "wvT": np.ascontiguousarray(Wv[rs].T.astype(np.float16)),
            "bq": np.ascontiguousarray(bq[rs]),
            "bk": np.ascontiguousarray(bk[rs]),
            "bv": np.ascontiguousarray(bv[rs]),
            "amask": np.ascontiguousarray(attn_mask[b]).astype(np.int32),
            "dmT": dmT_all[b, h0:h0 + NH],
        })
    return in_maps


def assemble_out(results):
    out = np.empty((B, H_TOT, S, Dh), dtype=np.float32)
    for c in range(N_CORES):
        b = c // 4
        h0 = (c % 4) * NH
        out[b, h0:h0 + NH] = results[c]["out"]
    return out


def kernel(query, key, value, attn_mask, dropout_mask, Wq, bq, Wk, bk, Wv, bv,
           _trace=False):
    from concourse.bass_utils import run_bass_kernel_spmd

    nc = _get_program()
    in_maps = make_in_maps(
        np.asarray(query, dtype=np.float32),
        np.asarray(key, dtype=np.float32),
        np.asarray(value, dtype=np.float32),
        np.asarray(attn_mask),
        np.asarray(dropout_mask, dtype=np.float32),
        np.asarray(Wq, dtype=np.float32), np.asarray(bq, dtype=np.float32),
        np.asarray(Wk, dtype=np.float32), np.asarray(bk, dtype=np.float32),
        np.asarray(Wv, dtype=np.float32), np.asarray(bv, dtype=np.float32))
    kw = {}
    if _trace:
        import os, shutil
        td = os.path.abspath("trace_out")
        shutil.rmtree(td, ignore_errors=True)
        os.makedirs(td, exist_ok=True)
        kw["tmpdir"] = td
    res = run_bass_kernel_spmd(
        nc, in_maps, list(range(N_CORES)), trace=_trace, **kw)
    out = assemble_out(res.results)
    if _trace:
        _CACHE["last_results"] = res
    return out


# revision 16
# speedup vs baseline: 1.1181x; 1.1181x over previous
"""Trainium2 Bass kernel for nn_AttentionModel (dense transformer MHA fwd).

Reference math (per batch b):
  q = x_q @ Wq.T + bq ; k,v likewise     (S=2048, E=1024, H=16, Dh=64)
  scores = q @ k.T  (per head)
  scores[sk where attn_mask[b,sk]==0] = -inf
  attn = softmax(scores, -1) * dropout_mask[b,h]
  out = attn @ v                          -> (B, H, S, Dh)

Sharding: 8 cores = 2 batches x 4 head-groups (4 heads/core). Pure data
parallel SPMD, no collectives; host slices inputs and restacks outputs.

v3 dataflow (fully transposed attention, K=128 everywhere):
  - Host pre-transposes x/W (f16) and dm (bf16, [h, g, sk, 512] layout);
    halves HBM traffic, no device-side transposes of x/W/dm needed.
  - Projections produce qT2 head-PAIR tensors [128, S] (head a rows 0-63,
    head b rows 64-127) straight from PSUM, and per-head half-zero kT2
    tensors; scores^T = kT2_h (stationary, K=128: zero rows contribute 0)
    @ qT2_p (moving). K=128 avoids the half-rate small-K PE mode.
  - Key mask applied as per-partition bias in the exp activation (sk is
    the partition dim of scores^T): masked rows exp to exactly 0.
  - PDMT = EmT * dmT elementwise, split across DVE and Pool engines.
  - attn@v accumulates out^T[d, sq] over sk tiles (v natural stationary);
    Z = colsum(EmT) via a ones-column matmul PSUM chain; 1/Z applied to
    out^T via an fp32r broadcast matmul + one DVE mul. Output is written
    to DRAM transposed [Dh, S]; the host restores [S, Dh].
"""

import numpy as np

S = 2048
E = 1024
H_TOT = 16
NH = 4  # heads per core
Dh = 64
B = 2
N_CORES = 8
ST = S // 128  # 16 sk-tiles
ET = E // 128  # 8 e-tiles
SCH = 4  # s-chunks of 512 for the projection phase
GROUPS = 4  # sq groups of 512
EXP_SHIFT = -12.0  # exp(s + EXP_SHIFT)
MASK_BIG = 60000.0
DM_FIX = 1.0015650      # (1/0.9) / bf16(1/0.9): dm is cast to bf16 on host

_CACHE = {}


def _build_program():
    import concourse.bacc as bacc
    import concourse.bass as bass
    import concourse.mybir as mybir
    import concourse.tile as tile
    from concourse.masks import make_identity
    from contextlib import ExitStack

    dt = mybir.dt
    F32 = dt.float32
    F32R = dt.float32r
    F16 = dt.float16
    BF16 = dt.bfloat16
    I32 = dt.int32

    nc = bacc.Bacc("TRN2", target_bir_lowering=False, debug=False)

    # host-pretransposed inputs
    xq_d = nc.dram_tensor("xqT", [E, S], F16, kind="ExternalInput")
    xk_d = nc.dram_tensor("xkT", [E, S], F16, kind="ExternalInput")
    xv_d = nc.dram_tensor("xvT", [E, S], F16, kind="ExternalInput")
    wq_d = nc.dram_tensor("wqT", [E, NH * Dh], F16, kind="ExternalInput")
    wk_d = nc.dram_tensor("wkT", [E, NH * Dh], F16, kind="ExternalInput")
    wv_d = nc.dram_tensor("wvT", [E, NH * Dh], F16, kind="ExternalInput")
    bq_d = nc.dram_tensor("bq", [NH * Dh], F32, kind="ExternalInput")
    bk_d = nc.dram_tensor("bk", [NH * Dh], F32, kind="ExternalInput")
    bv_d = nc.dram_tensor("bv", [NH * Dh], F32, kind="ExternalInput")
    am_d = nc.dram_tensor("amask", [S], I32, kind="ExternalInput")
    # dmT[h, g, sk, j] = dm[h, g*512 + j, sk]
    dm_d = nc.dram_tensor("dmT", [NH, GROUPS, S, 512], BF16, kind="ExternalInput")
    out_d = nc.dram_tensor("out", [NH, S, Dh], F32, kind="ExternalOutput")

    def r32(ap):
        return ap.bitcast(F32R)

    with tile.TileContext(nc) as tc, ExitStack() as ctx:
        const_pool = ctx.enter_context(tc.tile_pool(name="const", bufs=1))

        # --- attn_mask -> per-partition exp bias columns mb16[p, i] ---
        m_colI = const_pool.tile([128, ST], I32)
        nc.sync.dma_start(m_colI[:], am_d[:].rearrange("(t p) -> p t", p=128))
        m_col = const_pool.tile([128, ST], F32)
        nc.vector.tensor_copy(m_col[:], m_colI[:])
        # mb16 = m*BIG + (EXP_SHIFT - BIG): 0 -> -BIG+shift, 1 -> shift
        mb16 = const_pool.tile([128, ST], F32)
        nc.scalar.activation(
            mb16[:], m_col[:], mybir.ActivationFunctionType.Copy,
            bias=EXP_SHIFT - MASK_BIG, scale=MASK_BIG,
        )

        # --- bias columns for q/k pair evac; bv broadcast row; ones ---
        bqp = []
        bkp = []
        for p in range(2):
            t = const_pool.tile([128, 1], F32, tag=f"bqp{p}", name=f"bqp{p}")
            nc.sync.dma_start(t[:], bq_d[p * 128:(p + 1) * 128].rearrange("(c o) -> c o", o=1))
            bqp.append(t)
            t = const_pool.tile([128, 1], F32, tag=f"bkp{p}", name=f"bkp{p}")
            nc.sync.dma_start(t[:], bk_d[p * 128:(p + 1) * 128].rearrange("(c o) -> c o", o=1))
            bkp.append(t)
        ones_row = const_pool.tile([1, 128], F32)
        nc.gpsimd.memset(ones_row[:], 1.0)
        ones_col = const_pool.tile([128, 1], BF16)
        nc.gpsimd.memset(ones_col[:], 1.0)
        one_1x1 = const_pool.tile([1, 1], F32)
        nc.gpsimd.memset(one_1x1[:], 1.0)
        ident = const_pool.tile([64, 64], F32)
        make_identity(nc, ident[:])
        bv_row = const_pool.tile([1, NH * Dh], F32)
        nc.sync.dma_start(bv_row[:], bv_d[:].rearrange("(o c) -> o c", o=1))
        bv_bc = const_pool.tile([128, NH * Dh], F32)

        with tc.tile_pool(name="ps_misc", bufs=1, space="PSUM") as ps_misc:
            bc_ps = ps_misc.tile([128, NH * Dh], F32)
            nc.tensor.matmul(bc_ps[:], ones_row[:], bv_row[:])
            nc.scalar.mul(bv_bc[:], bc_ps[:], DM_FIX)

        # --- persistent attention-phase tensors ---
        big_pool = ctx.enter_context(tc.tile_pool(name="big", bufs=1))
        # qT2[p]: head pair p, rows 0-63 = head 2p, rows 64-127 = head 2p+1
        qT2 = [big_pool.tile([128, S], F16, tag=f"qT2{p}", name=f"qT2{p}")
               for p in range(2)]
        # kT2[h]: [128, S] with only this head's 64 rows nonzero
        kT2 = [big_pool.tile([128, S], F16, tag=f"kT2{h}", name=f"kT2{h}")
               for h in range(NH)]
        v16 = big_pool.tile([128, ST, NH * Dh], BF16)

        # zero the unused halves of kT2 (once)
        for h in range(NH):
            if h % 2 == 0:
                nc.gpsimd.memset(kT2[h][64:128, :], 0.0)
            else:
                nc.gpsimd.memset(kT2[h][0:64, :], 0.0)

        # ============ Phase 2: projections ============
        with tc.tile_pool(name="wt", bufs=1) as wtp, \
             tc.tile_pool(name="xT", bufs=3) as xtp, \
             tc.tile_pool(name="ps_prj", bufs=2, space="PSUM") as ps_prj, \
             tc.tile_pool(name="ps_prv", bufs=2, space="PSUM") as ps_prv:

            wts = {}
            for name, w_d in (("q", wq_d), ("k", wk_d), ("v", wv_d)):
                wt = wtp.tile([128, ET, NH * Dh], F16, tag=f"wt_{name}",
                              name=f"wt_{name}")
                nc.gpsimd.dma_start(
                    wt[:], w_d[:].rearrange("(et p) m -> p et m", p=128))
                wts[name] = wt

            for tens, x_d in (("v", xv_d), ("k", xk_d), ("q", xq_d)):
                for sc in range(SCH):
                    xt_c = xtp.tile([128, ET, 512], F16, tag="xt")
                    nc.gpsimd.dma_start(
                        xt_c[:],
                        x_d[:, sc * 512:(sc + 1) * 512].rearrange(
                            "(et p) n -> p et n", p=128))
                    sl = slice(sc * 512, (sc + 1) * 512)

                    if tens in ("q", "k"):
                        wt = wts[tens]
                        bias = bqp if tens == "q" else bkp
                        for p in range(2):
                            pq = ps_prj.tile([128, 512], F32, tag="pqk")
                            for et in range(ET):
                                nc.tensor.matmul(
                                    pq[:],
                                    wt[:, et, p * 128:(p + 1) * 128],
                                    xt_c[:, et, :],
                                    start=(et == 0), stop=(et == ET - 1))
                            if tens == "q":
                                nc.scalar.activation(
                                    qT2[p][:, sl], pq[:],
                                    mybir.ActivationFunctionType.Identity,
                                    bias=bias[p][:])
                            else:
                                nc.scalar.activation(
                                    kT2[2 * p][0:64, sl], pq[0:64, :],
                                    mybir.ActivationFunctionType.Identity,
                                    bias=bias[p][0:64, :])
                                nc.scalar.activation(
                                    kT2[2 * p + 1][64:128, sl], pq[64:128, :],
                                    mybir.ActivationFunctionType.Identity,
                                    bias=bias[p][64:128, :])
                    else:
                        for st in range(4):
                            pv = ps_prv.tile([128, NH * Dh], F32, tag="pv")
                            for et in range(ET):
                                nc.tensor.matmul(
                                    pv[:],
                                    xt_c[:, et, st * 128:(st + 1) * 128],
                                    wts["v"][:, et, :],
                                    start=(et == 0), stop=(et == ET - 1))
                            nc.vector.scalar_tensor_tensor(
                                out=v16[:, sc * 4 + st, :], in0=pv[:],
                                scalar=DM_FIX, in1=bv_bc[:],
                                op0=mybir.AluOpType.mult,
                                op1=mybir.AluOpType.add)

        # ============ Phase 3: attention (transposed, K=128) ============
        with tc.tile_pool(name="dmT", bufs=4) as dmp, \
             tc.tile_pool(name="em", bufs=10) as emp, \
             tc.tile_pool(name="pdmt", bufs=8) as pdmtp, \
             tc.tile_pool(name="rz", bufs=4) as rzp, \
             tc.tile_pool(name="ot", bufs=2) as otp, \
             tc.tile_pool(name="ostage", bufs=2) as ostp, \
             tc.tile_pool(name="ps_s", bufs=4, space="PSUM") as ps_s, \
             tc.tile_pool(name="ps_av", bufs=2, space="PSUM") as ps_av, \
             tc.tile_pool(name="ps_z", bufs=2, space="PSUM") as ps_z:

            for h in range(NH):
                p = h // 2
                ost = ostp.tile([128, ST * Dh], F32, tag="ost")
                for gp in range(2):
                    subs = []
                    for gsub in range(2):
                        g = gp * 2 + gsub
                        dmt = dmp.tile([128, ST, 512], BF16, tag="dmt")
                        nc.sync.dma_start(
                            dmt[:],
                            dm_d[h, g].rearrange("(t p) j -> p t j", p=128))
                        av = ps_av.tile([64, 512], F32, tag="av")
                        zps = ps_z.tile([1, 512], F32, tag="zps")
                        subs.append((g, dmt, av, zps))

                    # two interleaved streams; av/z matmuls emitted one
                    # iteration behind the scores so their DVE/Scalar inputs
                    # are ready when the PE reaches them (no mid-stream waits)
                    pending = []

                    def flush_pending():
                        for av_, zps_, v_ap, pdmt_, em_, i_ in pending:
                            nc.tensor.matmul(
                                av_[:], v_ap, pdmt_[:],
                                start=(i_ == 0), stop=(i_ == ST - 1))
                            nc.tensor.matmul(
                                zps_[:], ones_col[:], em_[:],
                                start=(i_ == 0), stop=(i_ == ST - 1))
                        del pending[:]

                    for i in range(ST):
                        nxt = []
                        for g, dmt, av, zps in subs:
                            sp = ps_s.tile([128, 512], F32, tag="sps")
                            nc.tensor.matmul(
                                sp[:],
                                kT2[h][:, i * 128:(i + 1) * 128],
                                qT2[p][:, g * 512:(g + 1) * 512])
                            em = emp.tile([128, 512], BF16, tag="em")
                            nc.scalar.activation(
                                em[:], sp[:],
                                mybir.ActivationFunctionType.Exp,
                                bias=mb16[:, i:i + 1])
                            pdmt = pdmtp.tile([128, 512], BF16, tag="pdmt")
                            eng = nc.gpsimd if (i % 5) < 2 else nc.vector
                            eng.tensor_mul(pdmt[:], em[:], dmt[:, i, :])
                            nxt.append(
                                (av, zps, v16[:, i, h * Dh:(h + 1) * Dh],
                                 pdmt, em, i))
                        flush_pending()
                        pending.extend(nxt)
                    flush_pending()

                    for g, dmt, av, zps in subs:
                        # Z row -> column form [128, 4] via K=1 matmuls that
                        # accumulate into one zeroed PSUM region
                        zrowS = rzp.tile([1, 512], F32, tag="zrowS")
                        nc.vector.tensor_copy(zrowS[:], zps[:])
                        zc4 = ps_s.tile([128, 512], F32, tag="sps")
                        for c in range(4):
                            nc.tensor.matmul(
                                zc4[:, c:c + 1],
                                zrowS[0:1, c * 128:(c + 1) * 128], one_1x1[:],
                                start=(c == 0), stop=(c == 3),
                                skip_group_check=True)
                        rz4 = rzp.tile([128, 4], F32, tag="rz4")
                        nc.vector.reciprocal(rz4[:], zc4[:, 0:4])
                        # out^T -> natural + 1/Z scale
                        ot64 = otp.tile([64, 512], F32, tag="ot64")
                        nc.vector.tensor_copy(ot64[:], av[:])
                        on4 = ps_s.tile([128, 512], F32, tag="sps")
                        for c in range(4):
                            nc.tensor.matmul(
                                on4[:, c * 64:(c + 1) * 64],
                                ot64[:, c * 128:(c + 1) * 128], ident[:],
                                is_transpose=True,
                                start=(c == 0), stop=(c == 3),
                                skip_group_check=True)
                        for c in range(4):
                            nc.vector.tensor_scalar_mul(
                                ost[:, g * 256 + c * 64:g * 256 + (c + 1) * 64],
                                on4[:, c * 64:(c + 1) * 64], rz4[:, c:c + 1])
                nc.sync.dma_start(
                    out_d[h].rearrange("(t p) d -> p t d", p=128), ost[:])

    nc.compile()
    return nc


def _get_program():
    if "nc" not in _CACHE:
        _CACHE["nc"] = _build_program()
    return _CACHE["nc"]


def make_in_maps(query, key, value, attn_mask, dropout_mask, Wq, bq, Wk, bk, Wv, bv):
    import ml_dtypes
    BF = ml_dtypes.bfloat16

    xT = {}
    for b in range(B):
        xT[("q", b)] = np.ascontiguousarray(query[b].T.astype(np.float16))
        xT[("k", b)] = np.ascontiguousarray(key[b].T.astype(np.float16))
        xT[("v", b)] = np.ascontiguousarray(value[b].T.astype(np.float16))
    # dmT[b][h, g, sk, j] = dm[b, h, g*512+j, sk]
    dmT_all = np.ascontiguousarray(
        dropout_mask.reshape(B, H_TOT, GROUPS, 512, S).transpose(0, 1, 2, 4, 3)
        .astype(BF))

    in_maps = []
    for c in range(N_CORES):
        b = c // 4
        h0 = (c % 4) * NH
        rs = slice(h0 * Dh, (h0 + NH) * Dh)
        in_maps.append({
            "xqT": xT[("q", b)],
            "xkT": xT[("k", b)],
            "xvT": xT[("v", b)],
            "wqT": np.ascontiguousarray(Wq[rs].T.astype(np.float16)),
            "wkT": np.ascontiguousarray(Wk[rs].T.astype(np.float16)),
            "wvT": np.ascontiguousarray(Wv[rs].T.astype(np.float16)),
            "bq": np.ascontiguousarray(bq[rs]),
            "bk": np.ascontiguousarray(bk[rs]),
            "bv": np.ascontiguousarray(bv[rs]),
            "amask": np.ascontiguousarray(attn_mask[b]).astype(np.int32),
            "dmT": dmT_all[b, h0:h0 + NH],
        })
    return in_maps


def assemble_out(results):
    out = np.empty((B, H_TOT, S, Dh), dtype=np.float32)
    for c in range(N_CORES):
        b = c // 4
        h0 = (c % 4) * NH
        out[b, h0:h0 + NH] = results[c]["out"]
    return out


def kernel(query, key, value, attn_mask, dropout_mask, Wq, bq, Wk, bk, Wv, bv,
           _trace=False):
    from concourse.bass_utils import run_bass_kernel_spmd

    nc = _get_program()
    in_maps = make_in_maps(
        np.asarray(query, dtype=np.float32),
        np.asarray(key, dtype=np.float32),
        np.asarray(value, dtype=np.float32),
        np.asarray(attn_mask),
        np.asarray(dropout_mask, dtype=np.float32),
        np.asarray(Wq, dtype=np.float32), np.asarray(bq, dtype=np.float32),
        np.asarray(Wk, dtype=np.float32), np.asarray(bk, dtype=np.float32),
        np.asarray(Wv, dtype=np.float32), np.asarray(bv, dtype=np.float32))
    kw = {}
    if _trace:
        import os, shutil
        td = os.path.abspath("trace_out")
        shutil.rmtree(td, ignore_errors=True)
        os.makedirs(td, exist_ok=True)
        kw["tmpdir"] = td
    res = run_bass_kernel_spmd(
        nc, in_maps, list(range(N_CORES)), trace=_trace, **kw)
    out = assemble_out(res.results)
    if _trace:
        _CACHE["last_results"] = res
    return out


# revision 17
# speedup vs baseline: 1.1336x; 1.0139x over previous
"""Trainium2 Bass kernel for nn_AttentionModel (dense transformer MHA fwd).

Reference math (per batch b):
  q = x_q @ Wq.T + bq ; k,v likewise     (S=2048, E=1024, H=16, Dh=64)
  scores = q @ k.T  (per head)
  scores[sk where attn_mask[b,sk]==0] = -inf
  attn = softmax(scores, -1) * dropout_mask[b,h]
  out = attn @ v                          -> (B, H, S, Dh)

Sharding: 8 cores = 2 batches x 4 head-groups (4 heads/core). Pure data
parallel SPMD, no collectives; host slices inputs and restacks outputs.

v3 dataflow (fully transposed attention, K=128 everywhere):
  - Host pre-transposes x/W (f16) and dm (bf16, [h, g, sk, 512] layout);
    halves HBM traffic, no device-side transposes of x/W/dm needed.
  - Projections produce qT2 head-PAIR tensors [128, S] (head a rows 0-63,
    head b rows 64-127) straight from PSUM, and per-head half-zero kT2
    tensors; scores^T = kT2_h (stationary, K=128: zero rows contribute 0)
    @ qT2_p (moving). K=128 avoids the half-rate small-K PE mode.
  - Key mask applied as per-partition bias in the exp activation (sk is
    the partition dim of scores^T): masked rows exp to exactly 0.
  - PDMT = EmT * dmT elementwise, split across DVE and Pool engines.
  - attn@v accumulates out^T[d, sq] over sk tiles (v natural stationary);
    Z = colsum(EmT) via a ones-column matmul PSUM chain; 1/Z applied to
    out^T via an fp32r broadcast matmul + one DVE mul. Output is written
    to DRAM transposed [Dh, S]; the host restores [S, Dh].
"""

import numpy as np

S = 2048
E = 1024
H_TOT = 16
NH = 4  # heads per core
Dh = 64
B = 2
N_CORES = 8
ST = S // 128  # 16 sk-tiles
ET = E // 128  # 8 e-tiles
SCH = 4  # s-chunks of 512 for the projection phase
GROUPS = 4  # sq groups of 512
EXP_SHIFT = -12.0  # exp(s + EXP_SHIFT)
MASK_BIG = 60000.0
DM_FIX = 1.0015650      # (1/0.9) / bf16(1/0.9): dm is cast to bf16 on host

_CACHE = {}


def _build_program():
    import concourse.bacc as bacc
    import concourse.bass as bass
    import concourse.mybir as mybir
    import concourse.tile as tile
    from concourse.masks import make_identity
    from contextlib import ExitStack

    dt = mybir.dt
    F32 = dt.float32
    F32R = dt.float32r
    F16 = dt.float16
    BF16 = dt.bfloat16
    I32 = dt.int32

    nc = bacc.Bacc("TRN2", target_bir_lowering=False, debug=False)

    # host-pretransposed inputs
    xq_d = nc.dram_tensor("xqT", [E, S], F16, kind="ExternalInput")
    xk_d = nc.dram_tensor("xkT", [E, S], F16, kind="ExternalInput")
    xv_d = nc.dram_tensor("xvT", [E, S], F16, kind="ExternalInput")
    wq_d = nc.dram_tensor("wqT", [E, NH * Dh], F16, kind="ExternalInput")
    wk_d = nc.dram_tensor("wkT", [E, NH * Dh], F16, kind="ExternalInput")
    wv_d = nc.dram_tensor("wvT", [E, NH * Dh], F16, kind="ExternalInput")
    bq_d = nc.dram_tensor("bq", [NH * Dh], F32, kind="ExternalInput")
    bk_d = nc.dram_tensor("bk", [NH * Dh], F32, kind="ExternalInput")
    bv_d = nc.dram_tensor("bv", [NH * Dh], F32, kind="ExternalInput")
    am_d = nc.dram_tensor("amask", [S], I32, kind="ExternalInput")
    # dmT[h, g, sk, j] = dm[h, g*512 + j, sk]
    dm_d = nc.dram_tensor("dmT", [NH, GROUPS, S, 512], BF16, kind="ExternalInput")
    out_d = nc.dram_tensor("out", [NH, S, Dh], F32, kind="ExternalOutput")

    def r32(ap):
        return ap.bitcast(F32R)

    with tile.TileContext(nc) as tc, ExitStack() as ctx:
        const_pool = ctx.enter_context(tc.tile_pool(name="const", bufs=1))

        # --- attn_mask -> per-partition exp bias columns mb16[p, i] ---
        m_colI = const_pool.tile([128, ST], I32)
        nc.sync.dma_start(m_colI[:], am_d[:].rearrange("(t p) -> p t", p=128))
        m_col = const_pool.tile([128, ST], F32)
        nc.vector.tensor_copy(m_col[:], m_colI[:])
        # mb16 = m*BIG + (EXP_SHIFT - BIG): 0 -> -BIG+shift, 1 -> shift
        mb16 = const_pool.tile([128, ST], F32)
        nc.scalar.activation(
            mb16[:], m_col[:], mybir.ActivationFunctionType.Copy,
            bias=EXP_SHIFT - MASK_BIG, scale=MASK_BIG,
        )

        # --- bias columns for q/k pair evac; bv broadcast row; ones ---
        bqp = []
        bkp = []
        for p in range(2):
            t = const_pool.tile([128, 1], F32, tag=f"bqp{p}", name=f"bqp{p}")
            nc.sync.dma_start(t[:], bq_d[p * 128:(p + 1) * 128].rearrange("(c o) -> c o", o=1))
            bqp.append(t)
            t = const_pool.tile([128, 1], F32, tag=f"bkp{p}", name=f"bkp{p}")
            nc.sync.dma_start(t[:], bk_d[p * 128:(p + 1) * 128].rearrange("(c o) -> c o", o=1))
            bkp.append(t)
        ones_row = const_pool.tile([1, 128], F32)
        nc.gpsimd.memset(ones_row[:], 1.0)
        ones_col = const_pool.tile([128, 1], BF16)
        nc.gpsimd.memset(ones_col[:], 1.0)
        one_1x1 = const_pool.tile([1, 1], F32)
        nc.gpsimd.memset(one_1x1[:], 1.0)
        ident = const_pool.tile([64, 64], F32)
        make_identity(nc, ident[:])
        bv_row = const_pool.tile([1, NH * Dh], F32)
        nc.sync.dma_start(bv_row[:], bv_d[:].rearrange("(o c) -> o c", o=1))
        bv_bc = const_pool.tile([128, NH * Dh], F32)

        with tc.tile_pool(name="ps_misc", bufs=1, space="PSUM") as ps_misc:
            bc_ps = ps_misc.tile([128, NH * Dh], F32)
            nc.tensor.matmul(bc_ps[:], ones_row[:], bv_row[:])
            nc.scalar.mul(bv_bc[:], bc_ps[:], DM_FIX)

        # --- persistent attention-phase tensors ---
        big_pool = ctx.enter_context(tc.tile_pool(name="big", bufs=1))
        # qT2[p]: head pair p, rows 0-63 = head 2p, rows 64-127 = head 2p+1
        qT2 = [big_pool.tile([128, S], F16, tag=f"qT2{p}", name=f"qT2{p}")
               for p in range(2)]
        # kT2[h]: [128, S] with only this head's 64 rows nonzero
        kT2 = [big_pool.tile([128, S], F16, tag=f"kT2{h}", name=f"kT2{h}")
               for h in range(NH)]
        v16 = big_pool.tile([128, ST, NH * Dh], BF16)

        # zero the unused halves of kT2 (once)
        for h in range(NH):
            if h % 2 == 0:
                nc.gpsimd.memset(kT2[h][64:128, :], 0.0)
            else:
                nc.gpsimd.memset(kT2[h][0:64, :], 0.0)

        # ============ Phase 2: projections ============
        with tc.tile_pool(name="wt", bufs=1) as wtp, \
             tc.tile_pool(name="xT", bufs=3) as xtp, \
             tc.tile_pool(name="ps_prj", bufs=2, space="PSUM") as ps_prj, \
             tc.tile_pool(name="ps_prv", bufs=2, space="PSUM") as ps_prv:

            wts = {}
            for name, w_d in (("q", wq_d), ("k", wk_d), ("v", wv_d)):
                wt = wtp.tile([128, ET, NH * Dh], F16, tag=f"wt_{name}",
                              name=f"wt_{name}")
                nc.gpsimd.dma_start(
                    wt[:], w_d[:].rearrange("(et p) m -> p et m", p=128))
                wts[name] = wt

            for tens, x_d in (("v", xv_d), ("k", xk_d), ("q", xq_d)):
                for sc in range(SCH):
                    xt_c = xtp.tile([128, ET, 512], F16, tag="xt")
                    nc.gpsimd.dma_start(
                        xt_c[:],
                        x_d[:, sc * 512:(sc + 1) * 512].rearrange(
                            "(et p) n -> p et n", p=128))
                    sl = slice(sc * 512, (sc + 1) * 512)

                    if tens in ("q", "k"):
                        wt = wts[tens]
                        bias = bqp if tens == "q" else bkp
                        for p in range(2):
                            pq = ps_prj.tile([128, 512], F32, tag="pqk")
                            for et in range(ET):
                                nc.tensor.matmul(
                                    pq[:],
                                    wt[:, et, p * 128:(p + 1) * 128],
                                    xt_c[:, et, :],
                                    start=(et == 0), stop=(et == ET - 1))
                            if tens == "q":
                                nc.scalar.activation(
                                    qT2[p][:, sl], pq[:],
                                    mybir.ActivationFunctionType.Identity,
                                    bias=bias[p][:])
                            else:
                                nc.scalar.activation(
                                    kT2[2 * p][0:64, sl], pq[0:64, :],
                                    mybir.ActivationFunctionType.Identity,
                                    bias=bias[p][0:64, :])
                                nc.scalar.activation(
                                    kT2[2 * p + 1][64:128, sl], pq[64:128, :],
                                    mybir.ActivationFunctionType.Identity,
                                    bias=bias[p][64:128, :])
                    else:
                        for st in range(4):
                            pv = ps_prv.tile([128, NH * Dh], F32, tag="pv")
                            for et in range(ET):
                                nc.tensor.matmul(
                                    pv[:],
                                    xt_c[:, et, st * 128:(st + 1) * 128],
                                    wts["v"][:, et, :],
                                    start=(et == 0), stop=(et == ET - 1))
                            nc.vector.scalar_tensor_tensor(
                                out=v16[:, sc * 4 + st, :], in0=pv[:],
                                scalar=DM_FIX, in1=bv_bc[:],
                                op0=mybir.AluOpType.mult,
                                op1=mybir.AluOpType.add)

        # ============ Phase 3: attention (transposed, K=128) ============
        with tc.tile_pool(name="dmT", bufs=4) as dmp, \
             tc.tile_pool(name="em", bufs=10) as emp, \
             tc.tile_pool(name="pdmt", bufs=8) as pdmtp, \
             tc.tile_pool(name="rz", bufs=4) as rzp, \
             tc.tile_pool(name="ot", bufs=2) as otp, \
             tc.tile_pool(name="ostage", bufs=2) as ostp, \
             tc.tile_pool(name="ps_s", bufs=4, space="PSUM") as ps_s, \
             tc.tile_pool(name="ps_av", bufs=2, space="PSUM") as ps_av, \
             tc.tile_pool(name="ps_z", bufs=2, space="PSUM") as ps_z:

            for h in range(NH):
                p = h // 2
                ost = ostp.tile([128, ST * Dh], F32, tag="ost")
                for gp in range(2):
                    subs = []
                    for gsub in range(2):
                        g = gp * 2 + gsub
                        dmt = dmp.tile([128, ST, 512], BF16, tag="dmt")
                        nc.sync.dma_start(
                            dmt[:],
                            dm_d[h, g].rearrange("(t p) j -> p t j", p=128))
                        av = ps_av.tile([64, 512], F32, tag="av")
                        zps = ps_z.tile([1, 512], F32, tag="zps")
                        subs.append((g, dmt, av, zps))

                    # two interleaved dependency streams keep the PE fed
                    for i in range(ST):
                        for g, dmt, av, zps in subs:
                            sp = ps_s.tile([128, 512], F32, tag="sps")
                            nc.tensor.matmul(
                                sp[:],
                                kT2[h][:, i * 128:(i + 1) * 128],
                                qT2[p][:, g * 512:(g + 1) * 512])
                            em = emp.tile([128, 512], BF16, tag="em")
                            nc.scalar.activation(
                                em[:], sp[:],
                                mybir.ActivationFunctionType.Exp,
                                bias=mb16[:, i:i + 1])
                            pdmt = pdmtp.tile([128, 512], BF16, tag="pdmt")
                            eng = nc.gpsimd if (i % 5) < 2 else nc.vector
                            eng.tensor_mul(pdmt[:], em[:], dmt[:, i, :])
                            nc.tensor.matmul(
                                av[:],
                                v16[:, i, h * Dh:(h + 1) * Dh],
                                pdmt[:],
                                start=(i == 0), stop=(i == ST - 1))
                            nc.tensor.matmul(
                                zps[:],
                                ones_col[:],
                                em[:],
                                start=(i == 0), stop=(i == ST - 1))

                    for g, dmt, av, zps in subs:
                        # Z row -> column form [128, 4] via K=1 matmuls that
                        # accumulate into one zeroed PSUM region
                        zrowS = rzp.tile([1, 512], F32, tag="zrowS")
                        nc.vector.tensor_copy(zrowS[:], zps[:])
                        zc4 = ps_s.tile([128, 512], F32, tag="sps")
                        for c in range(4):
                            nc.tensor.matmul(
                                zc4[:, c:c + 1],
                                zrowS[0:1, c * 128:(c + 1) * 128], one_1x1[:],
                                start=(c == 0), stop=(c == 3),
                                skip_group_check=True)
                        rz4 = rzp.tile([128, 4], F32, tag="rz4")
                        nc.vector.reciprocal(rz4[:], zc4[:, 0:4])
                        # out^T -> natural + 1/Z scale
                        ot64 = otp.tile([64, 512], F32, tag="ot64")
                        nc.vector.tensor_copy(ot64[:], av[:])
                        on4 = ps_s.tile([128, 512], F32, tag="sps")
                        for c in range(4):
                            nc.tensor.matmul(
                                on4[:, c * 64:(c + 1) * 64],
                                ot64[:, c * 128:(c + 1) * 128], ident[:],
                                is_transpose=True,
                                start=(c == 0), stop=(c == 3),
                                skip_group_check=True)
                        for c in range(4):
                            nc.vector.tensor_scalar_mul(
                                ost[:, g * 256 + c * 64:g * 256 + (c + 1) * 64],
                                on4[:, c * 64:(c + 1) * 64], rz4[:, c:c + 1])
                nc.sync.dma_start(
                    out_d[h].rearrange("(t p) d -> p t d", p=128), ost[:])

    nc.compile()
    return nc


def _get_program():
    if "nc" not in _CACHE:
        _CACHE["nc"] = _build_program()
    return _CACHE["nc"]


def make_in_maps(query, key, value, attn_mask, dropout_mask, Wq, bq, Wk, bk, Wv, bv):
    import ml_dtypes
    BF = ml_dtypes.bfloat16

    xT = {}
    for b in range(B):
        xT[("q", b)] = np.ascontiguousarray(query[b].T.astype(np.float16))
        xT[("k", b)] = np.ascontiguousarray(key[b].T.astype(np.float16))
        xT[("v", b)] = np.ascontiguousarray(value[b].T.astype(np.float16))
    # dmT[b][h, g, sk, j] = dm[b, h, g*512+j, sk]
    dmT_all = np.ascontiguousarray(
        dropout_mask.reshape(B, H_TOT, GROUPS, 512, S).transpose(0, 1, 2, 4, 3)
        .astype(BF))

    in_maps = []
    for c in range(N_CORES):
        b = c // 4
        h0 = (c % 4) * NH
        rs = slice(h0 * Dh, (h0 + NH) * Dh)
        in_maps.append({
            "xqT": xT[("q", b)],
            "xkT": xT[("k", b)],
            "xvT": xT[("v", b)],
            "wqT": np.ascontiguousarray(Wq[rs].T.astype(np.float16)),
            "wkT": np.ascontiguousarray(Wk[rs].T.astype(np.float16)),
            "wvT": np.ascontiguousarray(Wv[rs].T.astype(np.float16)),
            "bq": np.ascontiguousarray(bq[rs]),
            "bk": np.ascontiguousarray(bk[rs]),
            "bv": np.ascontiguousarray(bv[rs]),
            "amask": np.ascontiguousarray(attn_mask[b]).astype(np.int32),
            "dmT": dmT_all[b, h0:h0 + NH],
        })
    return in_maps


def assemble_out(results):
    out = np.empty((B, H_TOT, S, Dh), dtype=np.float32)
    for c in range(N_CORES):
        b = c // 4
        h0 = (c % 4) * NH
        out[b, h0:h0 + NH] = results[c]["out"]
    return out


def kernel(query, key, value, attn_mask, dropout_mask, Wq, bq, Wk, bk, Wv, bv,
           _trace=False):
    from concourse.bass_utils import run_bass_kernel_spmd

    nc = _get_program()
    in_maps = make_in_maps(
        np.asarray(query, dtype=np.float32),
        np.asarray(key, dtype=np.float32),
        np.asarray(value, dtype=np.float32),
        np.asarray(attn_mask),
        np.asarray(dropout_mask, dtype=np.float32),
        np.asarray(Wq, dtype=np.float32), np.asarray(bq, dtype=np.float32),
        np.asarray(Wk, dtype=np.float32), np.asarray(bk, dtype=np.float32),
        np.asarray(Wv, dtype=np.float32), np.asarray(bv, dtype=np.float32))
    kw = {}
    if _trace:
        import os, shutil
        td = os.path.abspath("trace_out")
        shutil.rmtree(td, ignore_errors=True)
        os.makedirs(td, exist_ok=True)
        kw["tmpdir"] = td
    res = run_bass_kernel_spmd(
        nc, in_maps, list(range(N_CORES)), trace=_trace, **kw)
    out = assemble_out(res.results)
    if _trace:
        _CACHE["last_results"] = res
    return out


# revision 19
# speedup vs baseline: 1.2010x; 1.0594x over previous
"""Trainium2 Bass kernel for nn_AttentionModel (dense transformer MHA fwd).

Reference math (per batch b):
  q = x_q @ Wq.T + bq ; k,v likewise     (S=2048, E=1024, H=16, Dh=64)
  scores = q @ k.T  (per head)
  scores[sk where attn_mask[b,sk]==0] = -inf
  attn = softmax(scores, -1) * dropout_mask[b,h]
  out = attn @ v                          -> (B, H, S, Dh)

Sharding: 8 cores = 2 batches x 4 head-groups (4 heads/core). Pure data
parallel SPMD, no collectives; host slices inputs and restacks outputs.

v3 dataflow (fully transposed attention, K=128 everywhere):
  - Host pre-transposes x/W (f16) and dm (bf16, [h, g, sk, 512] layout);
    halves HBM traffic, no device-side transposes of x/W/dm needed.
  - Projections produce qT2 head-PAIR tensors [128, S] (head a rows 0-63,
    head b rows 64-127) straight from PSUM, and per-head half-zero kT2
    tensors; scores^T = kT2_h (stationary, K=128: zero rows contribute 0)
    @ qT2_p (moving). K=128 avoids the half-rate small-K PE mode.
  - Key mask applied as per-partition bias in the exp activation (sk is
    the partition dim of scores^T): masked rows exp to exactly 0.
  - PDMT = EmT * dmT elementwise, split across DVE and Pool engines.
  - attn@v accumulates out^T[d, sq] over sk tiles (v natural stationary);
    Z = colsum(EmT) via a ones-column matmul PSUM chain; 1/Z applied to
    out^T via an fp32r broadcast matmul + one DVE mul. Output is written
    to DRAM transposed [Dh, S]; the host restores [S, Dh].
"""

import numpy as np

S = 2048
E = 1024
H_TOT = 16
NH = 4  # heads per core
Dh = 64
B = 2
N_CORES = 8
ST = S // 128  # 16 sk-tiles
ET = E // 128  # 8 e-tiles
SCH = 4  # s-chunks of 512 for the projection phase
GROUPS = 4  # sq groups of 512
EXP_SHIFT = -12.0  # exp(s + EXP_SHIFT)
MASK_BIG = 60000.0
DM_FIX = 1.0015650      # (1/0.9) / bf16(1/0.9): dm is cast to bf16 on host

_CACHE = {}


def _build_program():
    import concourse.bacc as bacc
    import concourse.bass as bass
    import concourse.mybir as mybir
    import concourse.tile as tile
    from concourse.masks import make_identity
    from contextlib import ExitStack

    dt = mybir.dt
    F32 = dt.float32
    F32R = dt.float32r
    F16 = dt.float16
    BF16 = dt.bfloat16
    I32 = dt.int32

    nc = bacc.Bacc("TRN2", target_bir_lowering=False, debug=False)

    # host-pretransposed inputs
    xq_d = nc.dram_tensor("xqT", [E, S], F16, kind="ExternalInput")
    xk_d = nc.dram_tensor("xkT", [E, S], F16, kind="ExternalInput")
    xv_d = nc.dram_tensor("xvT", [E, S], F16, kind="ExternalInput")
    wq_d = nc.dram_tensor("wqT", [E, NH * Dh], F16, kind="ExternalInput")
    wk_d = nc.dram_tensor("wkT", [E, NH * Dh], F16, kind="ExternalInput")
    wv_d = nc.dram_tensor("wvT", [E, NH * Dh], F16, kind="ExternalInput")
    bq_d = nc.dram_tensor("bq", [NH * Dh], F32, kind="ExternalInput")
    bk_d = nc.dram_tensor("bk", [NH * Dh], F32, kind="ExternalInput")
    bv_d = nc.dram_tensor("bv", [NH * Dh], F32, kind="ExternalInput")
    am_d = nc.dram_tensor("amask", [S], I32, kind="ExternalInput")
    # dmT[h, g, sk, j] = dm[h, g*512 + j, sk]
    dm_d = nc.dram_tensor("dmT", [NH, GROUPS, S, 512], BF16, kind="ExternalInput")
    out_d = nc.dram_tensor("out", [NH, S, Dh], F32, kind="ExternalOutput")

    def r32(ap):
        return ap.bitcast(F32R)

    with tile.TileContext(nc) as tc, ExitStack() as ctx:
        const_pool = ctx.enter_context(tc.tile_pool(name="const", bufs=1))

        # --- attn_mask -> per-partition exp bias columns mb16[p, i] ---
        m_colI = const_pool.tile([128, ST], I32)
        nc.sync.dma_start(m_colI[:], am_d[:].rearrange("(t p) -> p t", p=128))
        m_col = const_pool.tile([128, ST], F32)
        nc.vector.tensor_copy(m_col[:], m_colI[:])
        # mb16 = m*BIG + (EXP_SHIFT - BIG): 0 -> -BIG+shift, 1 -> shift
        mb16 = const_pool.tile([128, ST], F32)
        nc.scalar.activation(
            mb16[:], m_col[:], mybir.ActivationFunctionType.Copy,
            bias=EXP_SHIFT - MASK_BIG, scale=MASK_BIG,
        )

        # --- bias columns for q/k pair evac; bv broadcast row; ones ---
        bqp = []
        bkp = []
        for p in range(2):
            t = const_pool.tile([128, 1], F32, tag=f"bqp{p}", name=f"bqp{p}")
            nc.sync.dma_start(t[:], bq_d[p * 128:(p + 1) * 128].rearrange("(c o) -> c o", o=1))
            bqp.append(t)
            t = const_pool.tile([128, 1], F32, tag=f"bkp{p}", name=f"bkp{p}")
            nc.sync.dma_start(t[:], bk_d[p * 128:(p + 1) * 128].rearrange("(c o) -> c o", o=1))
            bkp.append(t)
        ones_row = const_pool.tile([1, 128], F32)
        nc.gpsimd.memset(ones_row[:], 1.0)
        ones_col = const_pool.tile([128, 1], BF16)
        nc.gpsimd.memset(ones_col[:], 1.0)
        one_1x1 = const_pool.tile([1, 1], F32)
        nc.gpsimd.memset(one_1x1[:], 1.0)
        ident = const_pool.tile([64, 64], F32)
        make_identity(nc, ident[:])
        bv_row = const_pool.tile([1, NH * Dh], F32)
        nc.sync.dma_start(bv_row[:], bv_d[:].rearrange("(o c) -> o c", o=1))
        bv_bc = const_pool.tile([128, NH * Dh], F32)

        with tc.tile_pool(name="ps_misc", bufs=1, space="PSUM") as ps_misc:
            bc_ps = ps_misc.tile([128, NH * Dh], F32)
            nc.tensor.matmul(bc_ps[:], ones_row[:], bv_row[:])
            nc.scalar.mul(bv_bc[:], bc_ps[:], DM_FIX)

        # --- persistent attention-phase tensors ---
        big_pool = ctx.enter_context(tc.tile_pool(name="big", bufs=1))
        # qT2[p]: head pair p, rows 0-63 = head 2p, rows 64-127 = head 2p+1
        qT2 = [big_pool.tile([128, S], F16, tag=f"qT2{p}", name=f"qT2{p}")
               for p in range(2)]
        # kT2[h]: [128, S] with only this head's 64 rows nonzero
        kT2 = [big_pool.tile([128, S], F16, tag=f"kT2{h}", name=f"kT2{h}")
               for h in range(NH)]
        v16 = big_pool.tile([128, ST, NH * Dh], BF16)

        # zero the unused halves of kT2 (once)
        for h in range(NH):
            if h % 2 == 0:
                nc.gpsimd.memset(kT2[h][64:128, :], 0.0)
            else:
                nc.gpsimd.memset(kT2[h][0:64, :], 0.0)

        dmp = ctx.enter_context(tc.tile_pool(name="dmT", bufs=5))

        def load_dmt(h, g):
            dmt = dmp.tile([128, ST, 512], BF16, tag="dmt")
            nc.sync.dma_start(
                dmt[:], dm_d[h, g].rearrange("(t p) j -> p t j", p=128))
            return dmt

        dm_pre = [load_dmt(0, g) for g in range(2)]

        # ============ Phase 2: projections ============
        with tc.tile_pool(name="wt", bufs=1) as wtp, \
             tc.tile_pool(name="xT", bufs=3) as xtp, \
             tc.tile_pool(name="ps_prj", bufs=2, space="PSUM") as ps_prj, \
             tc.tile_pool(name="ps_prv", bufs=2, space="PSUM") as ps_prv:

            wts = {}
            for name, w_d in (("q", wq_d), ("k", wk_d), ("v", wv_d)):
                wt = wtp.tile([128, ET, NH * Dh], F16, tag=f"wt_{name}",
                              name=f"wt_{name}")
                nc.gpsimd.dma_start(
                    wt[:], w_d[:].rearrange("(et p) m -> p et m", p=128))
                wts[name] = wt

            for tens, x_d in (("v", xv_d), ("k", xk_d), ("q", xq_d)):
                for sc in range(SCH):
                    xt_c = xtp.tile([128, ET, 512], F16, tag="xt")
                    nc.gpsimd.dma_start(
                        xt_c[:],
                        x_d[:, sc * 512:(sc + 1) * 512].rearrange(
                            "(et p) n -> p et n", p=128))
                    sl = slice(sc * 512, (sc + 1) * 512)

                    if tens in ("q", "k"):
                        wt = wts[tens]
                        bias = bqp if tens == "q" else bkp
                        for p in range(2):
                            pq = ps_prj.tile([128, 512], F32, tag="pqk")
                            for et in range(ET):
                                nc.tensor.matmul(
                                    pq[:],
                                    wt[:, et, p * 128:(p + 1) * 128],
                                    xt_c[:, et, :],
                                    start=(et == 0), stop=(et == ET - 1))
                            if tens == "q":
                                nc.scalar.activation(
                                    qT2[p][:, sl], pq[:],
                                    mybir.ActivationFunctionType.Identity,
                                    bias=bias[p][:])
                            else:
                                nc.scalar.activation(
                                    kT2[2 * p][0:64, sl], pq[0:64, :],
                                    mybir.ActivationFunctionType.Identity,
                                    bias=bias[p][0:64, :])
                                nc.scalar.activation(
                                    kT2[2 * p + 1][64:128, sl], pq[64:128, :],
                                    mybir.ActivationFunctionType.Identity,
                                    bias=bias[p][64:128, :])
                    else:
                        for st in range(4):
                            pv = ps_prv.tile([128, NH * Dh], F32, tag="pv")
                            for et in range(ET):
                                nc.tensor.matmul(
                                    pv[:],
                                    xt_c[:, et, st * 128:(st + 1) * 128],
                                    wts["v"][:, et, :],
                                    start=(et == 0), stop=(et == ET - 1))
                            nc.vector.scalar_tensor_tensor(
                                out=v16[:, sc * 4 + st, :], in0=pv[:],
                                scalar=DM_FIX, in1=bv_bc[:],
                                op0=mybir.AluOpType.mult,
                                op1=mybir.AluOpType.add)

        # ============ Phase 3: attention (transposed, K=128) ============
        with tc.tile_pool(name="em", bufs=12) as emp, \
             tc.tile_pool(name="ztree", bufs=8) as ztp, \
             tc.tile_pool(name="pdmt", bufs=8) as pdmtp, \
             tc.tile_pool(name="rz", bufs=4) as rzp, \
             tc.tile_pool(name="ot", bufs=2) as otp, \
             tc.tile_pool(name="ostage", bufs=2) as ostp, \
             tc.tile_pool(name="ps_s", bufs=4, space="PSUM") as ps_s, \
             tc.tile_pool(name="ps_av", bufs=2, space="PSUM") as ps_av, \
             tc.tile_pool(name="ps_z", bufs=2, space="PSUM") as ps_z:

            for h in range(NH):
                p = h // 2
                ost = ostp.tile([128, ST * Dh], F32, tag="ost")
                for gp in range(2):
                    subs = []
                    for gsub in range(2):
                        g = gp * 2 + gsub
                        if h == 0 and gp == 0:
                            dmt = dm_pre[gsub]
                        else:
                            dmt = load_dmt(h, g)
                        av = ps_av.tile([64, 512], F32, tag="av")
                        # stream 0: Z on PE chain; stream 1: Pool cascade
                        zps = None
                        if gsub == 0:
                            zps = ps_z.tile([1, 512], F32, tag="zps",
                                            name="zps")
                        subs.append([g, dmt, av, zps, [None] * 5])

                    for i in range(ST):
                        for g, dmt, av, zps, pend in subs:
                            sp = ps_s.tile([128, 512], F32, tag="sps")
                            nc.tensor.matmul(
                                sp[:],
                                kT2[h][:, i * 128:(i + 1) * 128],
                                qT2[p][:, g * 512:(g + 1) * 512])
                            em = emp.tile([128, 512], BF16, tag="em")
                            nc.scalar.activation(
                                em[:], sp[:],
                                mybir.ActivationFunctionType.Exp,
                                bias=mb16[:, i:i + 1])
                            pdmt = pdmtp.tile([128, 512], BF16, tag="pdmt")
                            nc.vector.tensor_mul(pdmt[:], em[:], dmt[:, i, :])
                            nc.tensor.matmul(
                                av[:],
                                v16[:, i, h * Dh:(h + 1) * Dh],
                                pdmt[:],
                                start=(i == 0), stop=(i == ST - 1))
                            if zps is not None:
                                nc.tensor.matmul(
                                    zps[:],
                                    ones_col[:],
                                    em[:],
                                    start=(i == 0), stop=(i == ST - 1))
                            else:
                                node, lvl = em, 0
                                while pend[lvl] is not None:
                                    s = ztp.tile([128, 512], BF16, tag="zt")
                                    nc.gpsimd.tensor_add(
                                        s[:], pend[lvl][:], node[:])
                                    pend[lvl] = None
                                    node = s
                                    lvl += 1
                                pend[lvl] = node

                    for g, dmt, av, zps, pend in subs:
                        if zps is None:
                            zps = ps_z.tile([1, 512], F32, tag="zps",
                                            name="zpsB")
                            nc.tensor.matmul(zps[:], ones_col[:], pend[4][:])
                        # Z row -> column form [128, 4] via K=1 matmuls that
                        # accumulate into one zeroed PSUM region
                        zrowS = rzp.tile([1, 512], F32, tag="zrowS")
                        nc.vector.tensor_copy(zrowS[:], zps[:])
                        zc4 = ps_s.tile([128, 512], F32, tag="sps")
                        for c in range(4):
                            nc.tensor.matmul(
                                zc4[:, c:c + 1],
                                zrowS[0:1, c * 128:(c + 1) * 128], one_1x1[:],
                                start=(c == 0), stop=(c == 3),
                                skip_group_check=True)
                        rz4 = rzp.tile([128, 4], F32, tag="rz4")
                        nc.vector.reciprocal(rz4[:], zc4[:, 0:4])
                        # out^T -> natural + 1/Z scale
                        ot64 = otp.tile([64, 512], F32, tag="ot64")
                        nc.vector.tensor_copy(ot64[:], av[:])
                        on4 = ps_s.tile([128, 512], F32, tag="sps")
                        for c in range(4):
                            nc.tensor.matmul(
                                on4[:, c * 64:(c + 1) * 64],
                                ot64[:, c * 128:(c + 1) * 128], ident[:],
                                is_transpose=True,
                                start=(c == 0), stop=(c == 3),
                                skip_group_check=True)
                        for c in range(4):
                            nc.vector.tensor_scalar_mul(
                                ost[:, g * 256 + c * 64:g * 256 + (c + 1) * 64],
                                on4[:, c * 64:(c + 1) * 64], rz4[:, c:c + 1])
                nc.sync.dma_start(
                    out_d[h].rearrange("(t p) d -> p t d", p=128), ost[:])

    nc.compile()
    return nc


def _get_program():
    if "nc" not in _CACHE:
        _CACHE["nc"] = _build_program()
    return _CACHE["nc"]


def make_in_maps(query, key, value, attn_mask, dropout_mask, Wq, bq, Wk, bk, Wv, bv):
    import ml_dtypes
    BF = ml_dtypes.bfloat16

    xT = {}
    for b in range(B):
        xT[("q", b)] = np.ascontiguousarray(query[b].T.astype(np.float16))
        xT[("k", b)] = np.ascontiguousarray(key[b].T.astype(np.float16))
        xT[("v", b)] = np.ascontiguousarray(value[b].T.astype(np.float16))
    # dmT[b][h, g, sk, j] = dm[b, h, g*512+j, sk]
    dmT_all = np.ascontiguousarray(
        dropout_mask.reshape(B, H_TOT, GROUPS, 512, S).transpose(0, 1, 2, 4, 3)
        .astype(BF))

    in_maps = []
    for c in range(N_CORES):
        b = c // 4
        h0 = (c % 4) * NH
        rs = slice(h0 * Dh, (h0 + NH) * Dh)
        in_maps.append({
            "xqT": xT[("q", b)],
            "xkT": xT[("k", b)],
            "xvT": xT[("v", b)],
            "wqT": np.ascontiguousarray(Wq[rs].T.astype(np.float16)),
            "wkT": np.ascontiguousarray(Wk[rs].T.astype(np.float16)),
            "wvT": np.ascontiguousarray(Wv[rs].T.astype(np.float16)),
            "bq": np.ascontiguousarray(bq[rs]),
            "bk": np.ascontiguousarray(bk[rs]),
            "bv": np.ascontiguousarray(bv[rs]),
            "amask": np.ascontiguousarray(attn_mask[b]).astype(np.int32),
            "dmT": dmT_all[b, h0:h0 + NH],
        })
    return in_maps


def assemble_out(results):
    out = np.empty((B, H_TOT, S, Dh), dtype=np.float32)
    for c in range(N_CORES):
        b = c // 4
        h0 = (c % 4) * NH
        out[b, h0:h0 + NH] = results[c]["out"]
    return out


def kernel(query, key, value, attn_mask, dropout_mask, Wq, bq, Wk, bk, Wv, bv,
           _trace=False):
    from concourse.bass_utils import run_bass_kernel_spmd

    nc = _get_program()
    in_maps = make_in_maps(
        np.asarray(query, dtype=np.float32),
        np.asarray(key, dtype=np.float32),
        np.asarray(value, dtype=np.float32),
        np.asarray(attn_mask),
        np.asarray(dropout_mask, dtype=np.float32),
        np.asarray(Wq, dtype=np.float32), np.asarray(bq, dtype=np.float32),
        np.asarray(Wk, dtype=np.float32), np.asarray(bk, dtype=np.float32),
        np.asarray(Wv, dtype=np.float32), np.asarray(bv, dtype=np.float32))
    kw = {}
    if _trace:
        import os, shutil
        td = os.path.abspath("trace_out")
        shutil.rmtree(td, ignore_errors=True)
        os.makedirs(td, exist_ok=True)
        kw["tmpdir"] = td
    res = run_bass_kernel_spmd(
        nc, in_maps, list(range(N_CORES)), trace=_trace, **kw)
    out = assemble_out(res.results)
    if _trace:
        _CACHE["last_results"] = res
    return out


# revision 21
# speedup vs baseline: 1.2037x; 1.0023x over previous
"""Trainium2 Bass kernel for nn_AttentionModel (dense transformer MHA fwd).

Reference math (per batch b):
  q = x_q @ Wq.T + bq ; k,v likewise     (S=2048, E=1024, H=16, Dh=64)
  scores = q @ k.T  (per head)
  scores[sk where attn_mask[b,sk]==0] = -inf
  attn = softmax(scores, -1) * dropout_mask[b,h]
  out = attn @ v                          -> (B, H, S, Dh)

Sharding: 8 cores = 2 batches x 4 head-groups (4 heads/core). Pure data
parallel SPMD, no collectives; host slices inputs and restacks outputs.

v3 dataflow (fully transposed attention, K=128 everywhere):
  - Host pre-transposes x/W (f16) and dm (bf16, [h, g, sk, 512] layout);
    halves HBM traffic, no device-side transposes of x/W/dm needed.
  - Projections produce qT2 head-PAIR tensors [128, S] (head a rows 0-63,
    head b rows 64-127) straight from PSUM, and per-head half-zero kT2
    tensors; scores^T = kT2_h (stationary, K=128: zero rows contribute 0)
    @ qT2_p (moving). K=128 avoids the half-rate small-K PE mode.
  - Key mask applied as per-partition bias in the exp activation (sk is
    the partition dim of scores^T): masked rows exp to exactly 0.
  - PDMT = EmT * dmT elementwise, split across DVE and Pool engines.
  - attn@v accumulates out^T[d, sq] over sk tiles (v natural stationary);
    Z = colsum(EmT) via a ones-column matmul PSUM chain; 1/Z applied to
    out^T via an fp32r broadcast matmul + one DVE mul. Output is written
    to DRAM transposed [Dh, S]; the host restores [S, Dh].
"""

import numpy as np

S = 2048
E = 1024
H_TOT = 16
NH = 4  # heads per core
Dh = 64
B = 2
N_CORES = 8
ST = S // 128  # 16 sk-tiles
ET = E // 128  # 8 e-tiles
SCH = 4  # s-chunks of 512 for the projection phase
GROUPS = 4  # sq groups of 512
EXP_SHIFT = -12.0  # exp(s + EXP_SHIFT)
MASK_BIG = 60000.0
DM_FIX = 1.0015650      # (1/0.9) / bf16(1/0.9): dm is cast to bf16 on host

_CACHE = {}


def _build_program():
    import concourse.bacc as bacc
    import concourse.bass as bass
    import concourse.mybir as mybir
    import concourse.tile as tile
    from concourse.masks import make_identity
    from contextlib import ExitStack

    dt = mybir.dt
    F32 = dt.float32
    F32R = dt.float32r
    F16 = dt.float16
    BF16 = dt.bfloat16
    I32 = dt.int32

    nc = bacc.Bacc("TRN2", target_bir_lowering=False, debug=False)

    # host-pretransposed inputs
    xq_d = nc.dram_tensor("xqT", [E, S], F16, kind="ExternalInput")
    xk_d = nc.dram_tensor("xkT", [E, S], F16, kind="ExternalInput")
    xv_d = nc.dram_tensor("xvT", [E, S], F16, kind="ExternalInput")
    wq_d = nc.dram_tensor("wqT", [E, NH * Dh], F16, kind="ExternalInput")
    wk_d = nc.dram_tensor("wkT", [E, NH * Dh], F16, kind="ExternalInput")
    wv_d = nc.dram_tensor("wvT", [E, NH * Dh], F16, kind="ExternalInput")
    bq_d = nc.dram_tensor("bq", [NH * Dh], F32, kind="ExternalInput")
    bk_d = nc.dram_tensor("bk", [NH * Dh], F32, kind="ExternalInput")
    bv_d = nc.dram_tensor("bv", [NH * Dh], F32, kind="ExternalInput")
    am_d = nc.dram_tensor("amask", [S], I32, kind="ExternalInput")
    # dmT[h, g, sk, j] = dm[h, g*512 + j, sk]
    dm_d = nc.dram_tensor("dmT", [NH, GROUPS, S, 512], BF16, kind="ExternalInput")
    out_d = nc.dram_tensor("out", [NH, S, Dh], F32, kind="ExternalOutput")

    def r32(ap):
        return ap.bitcast(F32R)

    with tile.TileContext(nc) as tc, ExitStack() as ctx:
        const_pool = ctx.enter_context(tc.tile_pool(name="const", bufs=1))

        # --- attn_mask -> per-partition exp bias columns mb16[p, i] ---
        m_colI = const_pool.tile([128, ST], I32)
        nc.sync.dma_start(m_colI[:], am_d[:].rearrange("(t p) -> p t", p=128))
        m_col = const_pool.tile([128, ST], F32)
        nc.vector.tensor_copy(m_col[:], m_colI[:])
        # mb16 = m*BIG + (EXP_SHIFT - BIG): 0 -> -BIG+shift, 1 -> shift
        mb16 = const_pool.tile([128, ST], F32)
        nc.scalar.activation(
            mb16[:], m_col[:], mybir.ActivationFunctionType.Copy,
            bias=EXP_SHIFT - MASK_BIG, scale=MASK_BIG,
        )

        # --- bias columns for q/k pair evac; bv broadcast row; ones ---
        bqp = []
        bkp = []
        for p in range(2):
            t = const_pool.tile([128, 1], F32, tag=f"bqp{p}", name=f"bqp{p}")
            nc.sync.dma_start(t[:], bq_d[p * 128:(p + 1) * 128].rearrange("(c o) -> c o", o=1))
            bqp.append(t)
            t = const_pool.tile([128, 1], F32, tag=f"bkp{p}", name=f"bkp{p}")
            nc.sync.dma_start(t[:], bk_d[p * 128:(p + 1) * 128].rearrange("(c o) -> c o", o=1))
            bkp.append(t)
        ones_row = const_pool.tile([1, 128], F32)
        nc.gpsimd.memset(ones_row[:], 1.0)
        ones_col = const_pool.tile([128, 1], BF16)
        nc.gpsimd.memset(ones_col[:], 1.0)
        one_1x1 = const_pool.tile([1, 1], F32)
        nc.gpsimd.memset(one_1x1[:], 1.0)
        ident = const_pool.tile([64, 64], F32)
        make_identity(nc, ident[:])
        bv_row = const_pool.tile([1, NH * Dh], F32)
        nc.sync.dma_start(bv_row[:], bv_d[:].rearrange("(o c) -> o c", o=1))
        bv_bc = const_pool.tile([128, NH * Dh], F32)

        with tc.tile_pool(name="ps_misc", bufs=1, space="PSUM") as ps_misc:
            bc_ps = ps_misc.tile([128, NH * Dh], F32)
            nc.tensor.matmul(bc_ps[:], ones_row[:], bv_row[:])
            nc.scalar.mul(bv_bc[:], bc_ps[:], DM_FIX)

        # --- persistent attention-phase tensors ---
        big_pool = ctx.enter_context(tc.tile_pool(name="big", bufs=1))
        # qT2[p]: head pair p, rows 0-63 = head 2p, rows 64-127 = head 2p+1
        qT2 = [big_pool.tile([128, S], F16, tag=f"qT2{p}", name=f"qT2{p}")
               for p in range(2)]
        # kT2[h]: [128, S] with only this head's 64 rows nonzero
        kT2 = [big_pool.tile([128, S], F16, tag=f"kT2{h}", name=f"kT2{h}")
               for h in range(NH)]
        v16 = big_pool.tile([128, ST, NH * Dh], BF16)

        # zero the unused halves of kT2 (once)
        for h in range(NH):
            if h % 2 == 0:
                nc.gpsimd.memset(kT2[h][64:128, :], 0.0)
            else:
                nc.gpsimd.memset(kT2[h][0:64, :], 0.0)

        dmp = ctx.enter_context(tc.tile_pool(name="dmT", bufs=5))

        def load_dmt(h, g):
            dmt = dmp.tile([128, ST, 512], BF16, tag="dmt")
            nc.sync.dma_start(
                dmt[:], dm_d[h, g].rearrange("(t p) j -> p t j", p=128))
            return dmt

        dm_pre = [load_dmt(0, g) for g in range(2)]

        # ============ Phase 2: projections ============
        with tc.tile_pool(name="wt", bufs=1) as wtp, \
             tc.tile_pool(name="xT", bufs=3) as xtp, \
             tc.tile_pool(name="ps_prj", bufs=2, space="PSUM") as ps_prj, \
             tc.tile_pool(name="ps_prv", bufs=2, space="PSUM") as ps_prv:

            wts = {}
            for name, w_d in (("q", wq_d), ("k", wk_d), ("v", wv_d)):
                wt = wtp.tile([128, ET, NH * Dh], F16, tag=f"wt_{name}",
                              name=f"wt_{name}")
                nc.gpsimd.dma_start(
                    wt[:], w_d[:].rearrange("(et p) m -> p et m", p=128))
                wts[name] = wt

            for tens, x_d in (("v", xv_d), ("k", xk_d), ("q", xq_d)):
                for sc in range(SCH):
                    xt_c = xtp.tile([128, ET, 512], F16, tag="xt")
                    nc.gpsimd.dma_start(
                        xt_c[:],
                        x_d[:, sc * 512:(sc + 1) * 512].rearrange(
                            "(et p) n -> p et n", p=128))
                    sl = slice(sc * 512, (sc + 1) * 512)

                    if tens in ("q", "k"):
                        wt = wts[tens]
                        bias = bqp if tens == "q" else bkp
                        for p in range(2):
                            pq = ps_prj.tile([128, 512], F32, tag="pqk")
                            for et in range(ET):
                                nc.tensor.matmul(
                                    pq[:],
                                    wt[:, et, p * 128:(p + 1) * 128],
                                    xt_c[:, et, :],
                                    start=(et == 0), stop=(et == ET - 1))
                            if tens == "q":
                                nc.scalar.activation(
                                    qT2[p][:, sl], pq[:],
                                    mybir.ActivationFunctionType.Identity,
                                    bias=bias[p][:])
                            else:
                                nc.scalar.activation(
                                    kT2[2 * p][0:64, sl], pq[0:64, :],
                                    mybir.ActivationFunctionType.Identity,
                                    bias=bias[p][0:64, :])
                                nc.scalar.activation(
                                    kT2[2 * p + 1][64:128, sl], pq[64:128, :],
                                    mybir.ActivationFunctionType.Identity,
                                    bias=bias[p][64:128, :])
                    else:
                        for st in range(4):
                            pv = ps_prv.tile([128, NH * Dh], F32, tag="pv")
                            for et in range(ET):
                                nc.tensor.matmul(
                                    pv[:],
                                    xt_c[:, et, st * 128:(st + 1) * 128],
                                    wts["v"][:, et, :],
                                    start=(et == 0), stop=(et == ET - 1))
                            nc.vector.scalar_tensor_tensor(
                                out=v16[:, sc * 4 + st, :], in0=pv[:],
                                scalar=DM_FIX, in1=bv_bc[:],
                                op0=mybir.AluOpType.mult,
                                op1=mybir.AluOpType.add)

        # ============ Phase 3: attention (transposed, K=128) ============
        with tc.tile_pool(name="em", bufs=12) as emp, \
             tc.tile_pool(name="ztree", bufs=8) as ztp, \
             tc.tile_pool(name="pdmt", bufs=8) as pdmtp, \
             tc.tile_pool(name="rz", bufs=4) as rzp, \
             tc.tile_pool(name="ot", bufs=2) as otp, \
             tc.tile_pool(name="ostage", bufs=2) as ostp, \
             tc.tile_pool(name="ps_s", bufs=4, space="PSUM") as ps_s, \
             tc.tile_pool(name="ps_av", bufs=2, space="PSUM") as ps_av, \
             tc.tile_pool(name="ps_z", bufs=2, space="PSUM") as ps_z:

            for h in range(NH):
                p = h // 2
                ost = ostp.tile([128, ST * Dh], F32, tag="ost")
                for gp in range(2):
                    subs = []
                    for gsub in range(2):
                        g = gp * 2 + gsub
                        if h == 0 and gp == 0:
                            dmt = dm_pre[gsub]
                        else:
                            dmt = load_dmt(h, g)
                        av = ps_av.tile([64, 512], F32, tag="av")
                        # stream 0: Z on PE chain; stream 1: Pool cascade
                        zps = None
                        if gsub == 0:
                            zps = ps_z.tile([1, 512], F32, tag="zps",
                                            name="zps")
                        subs.append([g, dmt, av, zps, [None] * 5])

                    for i in range(ST):
                        for g, dmt, av, zps, pend in subs:
                            sp = ps_s.tile([128, 512], F32, tag="sps")
                            nc.tensor.matmul(
                                sp[:],
                                kT2[h][:, i * 128:(i + 1) * 128],
                                qT2[p][:, g * 512:(g + 1) * 512])
                            em = emp.tile([128, 512], BF16, tag="em")
                            nc.scalar.activation(
                                em[:], sp[:],
                                mybir.ActivationFunctionType.Exp,
                                bias=mb16[:, i:i + 1])
                            pdmt = pdmtp.tile([128, 512], BF16, tag="pdmt")
                            nc.vector.tensor_mul(pdmt[:], em[:], dmt[:, i, :])
                            nc.tensor.matmul(
                                av[:],
                                v16[:, i, h * Dh:(h + 1) * Dh],
                                pdmt[:],
                                start=(i == 0), stop=(i == ST - 1))
                            if zps is not None:
                                nc.tensor.matmul(
                                    zps[:],
                                    ones_col[:],
                                    em[:],
                                    start=(i == 0), stop=(i == ST - 1))
                            else:
                                node, lvl = em, 0
                                while pend[lvl] is not None:
                                    s = ztp.tile([128, 512], BF16, tag="zt")
                                    nc.gpsimd.tensor_add(
                                        s[:], pend[lvl][:], node[:])
                                    pend[lvl] = None
                                    node = s
                                    lvl += 1
                                pend[lvl] = node

                    for g, dmt, av, zps, pend in subs:
                        if zps is None:
                            zps = ps_z.tile([1, 512], F32, tag="zps",
                                            name="zpsB")
                            nc.tensor.matmul(zps[:], ones_col[:], pend[4][:])
                        # Z row -> column form [128, 4] via K=1 matmuls that
                        # accumulate into one zeroed PSUM region
                        zrowS = rzp.tile([1, 512], F32, tag="zrowS")
                        nc.vector.tensor_copy(zrowS[:], zps[:])
                        zc4 = ps_s.tile([128, 512], F32, tag="sps")
                        for c in range(4):
                            nc.tensor.matmul(
                                zc4[:, c:c + 1],
                                zrowS[0:1, c * 128:(c + 1) * 128], one_1x1[:],
                                start=(c == 0), stop=(c == 3),
                                skip_group_check=True)
                        rz4 = rzp.tile([128, 4], F32, tag="rz4")
                        nc.vector.reciprocal(rz4[:], zc4[:, 0:4])
                        # out^T -> natural + 1/Z scale
                        ot64 = otp.tile([64, 512], F32, tag="ot64")
                        nc.vector.tensor_copy(ot64[:], av[:])
                        on4 = ps_s.tile([128, 512], F32, tag="sps")
                        for c in range(4):
                            nc.tensor.matmul(
                                on4[:, c * 64:(c + 1) * 64],
                                ot64[:, c * 128:(c + 1) * 128], ident[:],
                                is_transpose=True,
                                start=(c == 0), stop=(c == 3),
                                skip_group_check=True)
                        for c in range(4):
                            nc.vector.tensor_scalar_mul(
                                ost[:, g * 256 + c * 64:g * 256 + (c + 1) * 64],
                                on4[:, c * 64:(c + 1) * 64], rz4[:, c:c + 1])
                nc.sync.dma_start(
                    out_d[h].rearrange("(t p) d -> p t d", p=128), ost[:])

    nc.compile()
    return nc


def _get_program():
    if "nc" not in _CACHE:
        _CACHE["nc"] = _build_program()
    return _CACHE["nc"]


def make_in_maps(query, key, value, attn_mask, dropout_mask, Wq, bq, Wk, bk, Wv, bv):
    import ml_dtypes
    BF = ml_dtypes.bfloat16

    xT = {}
    for b in range(B):
        xT[("q", b)] = np.ascontiguousarray(query[b].T.astype(np.float16))
        xT[("k", b)] = np.ascontiguousarray(key[b].T.astype(np.float16))
        xT[("v", b)] = np.ascontiguousarray(value[b].T.astype(np.float16))
    # dmT[b][h, g, sk, j] = dm[b, h, g*512+j, sk]
    dmT_all = np.ascontiguousarray(
        dropout_mask.reshape(B, H_TOT, GROUPS, 512, S).transpose(0, 1, 2, 4, 3)
        .astype(BF))

    in_maps = []
    for c in range(N_CORES):
        b = c // 4
        h0 = (c % 4) * NH
        rs = slice(h0 * Dh, (h0 + NH) * Dh)
        in_maps.append({
            "xqT": xT[("q", b)],
            "xkT": xT[("k", b)],
            "xvT": xT[("v", b)],
            "wqT": np.ascontiguousarray(Wq[rs].T.astype(np.float16)),
            "wkT": np.ascontiguousarray(Wk[rs].T.astype(np.float16)),
            "wvT": np.ascontiguousarray(Wv[rs].T.astype(np.float16)),
            "bq": np.ascontiguousarray(bq[rs]),
            "bk": np.ascontiguousarray(bk[rs]),
            "bv": np.ascontiguousarray(bv[rs]),
            "amask": np.ascontiguousarray(attn_mask[b]).astype(np.int32),
            "dmT": dmT_all[b, h0:h0 + NH],
        })
    return in_maps


def assemble_out(results):
    out = np.empty((B, H_TOT, S, Dh), dtype=np.float32)
    for c in range(N_CORES):
        b = c // 4
        h0 = (c % 4) * NH
        out[b, h0:h0 + NH] = results[c]["out"]
    return out


def kernel(query, key, value, attn_mask, dropout_mask, Wq, bq, Wk, bk, Wv, bv,
           _trace=False):
    from concourse.bass_utils import run_bass_kernel_spmd

    nc = _get_program()
    in_maps = make_in_maps(
        np.asarray(query, dtype=np.float32),
        np.asarray(key, dtype=np.float32),
        np.asarray(value, dtype=np.float32),
        np.asarray(attn_mask),
        np.asarray(dropout_mask, dtype=np.float32),
        np.asarray(Wq, dtype=np.float32), np.asarray(bq, dtype=np.float32),
        np.asarray(Wk, dtype=np.float32), np.asarray(bk, dtype=np.float32),
        np.asarray(Wv, dtype=np.float32), np.asarray(bv, dtype=np.float32))
    kw = {}
    if _trace:
        import os, shutil
        td = os.path.abspath("trace_out")
        shutil.rmtree(td, ignore_errors=True)
        os.makedirs(td, exist_ok=True)
        kw["tmpdir"] = td
    res = run_bass_kernel_spmd(
        nc, in_maps, list(range(N_CORES)), trace=_trace, **kw)
    out = assemble_out(res.results)
    if _trace:
        _CACHE["last_results"] = res
    return out
